# revision 1
# baseline (speedup 1.0000x reference)
"""Trainium2 Bass kernel for nn_DAG_61246233641129 (gnn_message_passing).

Math: sequential DAG over N=4224 nodes, out_j = tanh(x @ W[j,:1024] +
sum_{i<j} out_i * W[j,1024+i]); final output = sigmoid of last 128 nodes'
outputs, shape [512, 128].

Strategy (hardcoded, self-contained):
  * Data-parallel: batch 512 sharded 8 ways (64 rows/core), W replicated.
    Only the needed lower-block-triangle of W is packed (bf16, ~26MB/core;
    mostly-unused top rows of each panel trimmed into compact strips) so
    HBM traffic is near the useful-bytes floor (~76us/core) -- the kernel
    is DMA-bound, cost-model end-to-end ~93us.
  * Nodes in 33 blocks of 128; 4 blocks share a [64 batch, 512 node] PSUM
    bank. Cross-block/input contributions are PE matmuls with the small
    x/Y tile stationary and W streaming 512-wide, dripped between
    critical-path ops; panel chunks are DMA'd just-in-time by source
    availability so the post-DMA dependent tail stays short.
  * Per block the bank slice is copied+PE-transposed into TWO
    [128 node, 64 batch] work banks A and B, solving the intra-block
    recurrence y = tanh(base + L_strict @ y) by seeded fixed point as two
    decoupled one-ACT chains: y0 = tanh(partial base) runs a block early;
    A += W_prev @ y1_prev (stale) + L @ y0 -> y1 = tanh(A);
    B additionally patches W_prev @ (y2 - y1)_prev, += L @ y1 ->
    y2 = tanh(B) (exact base). HW-verified vs the jax reference:
    max abs err 4.3e-3, rms rel 1.3e-3 (bf16 quantization floor).
"""

import numpy as np
import ml_dtypes

BF16 = ml_dtypes.bfloat16

B = 512            # batch
IN = 1024          # input features
NN = 4224          # nodes
OUT = 128          # output nodes
NCORES = 8
BL = B // NCORES   # 64 batch rows per core
NB = 128           # node block
NBLK = NN // NB    # 33
KX = IN // 128     # 8 input k-tiles
GROUP = 4          # node blocks per [64, 512] PSUM bank
NGRP = (NBLK + GROUP - 1) // GROUP  # 9 (last group has 1 block)
CHUNK = 8          # k-tiles per DMA chunk of a panel (env K_CHUNK)
import os

LOOKAHEAD = int(os.environ.get("K_LOOKAHEAD", "8"))  # blocks of early group alloc
K_DRIP1 = int(os.environ.get("K_DRIP1", "1"))  # drip MMs inside the y1 window
K_DRIP2 = int(os.environ.get("K_DRIP2", "5"))  # drip MMs at end of block
K_MARGIN = int(os.environ.get("K_MARGIN", "3"))  # chunk DMA prefetch margin
K_PF = int(os.environ.get("K_PF", "10"))  # max blocks of early chunk DMA
K_BT = int(os.environ.get("K_BT", "4"))   # bt-bank buffers
K_WK = int(os.environ.get("K_WK", "4"))   # work-bank buffers
CHUNK = int(os.environ.get("K_CHUNK", str(CHUNK)))

_CACHE = {}


def _grp_cw(g):
    return 128 * min(GROUP, NBLK - GROUP * g)


def _grp_dmax(g):
    return min(GROUP * g + GROUP - 1, NBLK - 1)


def _grp_kt(g):
    return KX + _grp_dmax(g) + 1


def _grp_full(g):
    return _grp_cw(g) == 512


def _grp_ktm(g):
    """Main-panel rows: full groups push their last 3 (mostly unused) rows
    into a compact 'wd' strip; the last narrow group keeps everything."""
    return KX + GROUP * g + 1 if _grp_full(g) else _grp_kt(g)


def _grp_chunks(g):
    kt_n = _grp_ktm(g)
    return [(c0, min(c0 + CHUNK, kt_n)) for c0 in range(0, kt_n, CHUNK)]


# wd strip layout (full groups): [row KX+4g+1 cols 128:512 | row KX+4g+2
# cols 256:512 | row KX+4g+3 cols 384:512] -> local offsets 0/384/640, 768 wide
WD_W = 768


def _build_module():
    import concourse.mybir as mybir
    import concourse.tile as tile
    from concourse import bacc
    from concourse.bass import ds, ts
    from concourse.masks import make_identity
    from contextlib import ExitStack

    bf = mybir.dt.bfloat16
    f32 = mybir.dt.float32
    Tanh = mybir.ActivationFunctionType.Tanh
    Sigmoid = mybir.ActivationFunctionType.Sigmoid

    nc = bacc.Bacc()
    x_in = nc.dram_tensor("xt", [128, KX, BL], bf, kind="ExternalInput")
    w_in = {}
    wd_in = {}
    for g in range(NGRP):
        cw = _grp_cw(g)
        for ci, (k0, k1) in enumerate(_grp_chunks(g)):
            w_in[(g, ci)] = nc.dram_tensor(
                f"w{g}_{ci}", [128, k1 - k0, cw], bf, kind="ExternalInput"
            )
        if _grp_full(g):
            wd_in[g] = nc.dram_tensor(f"wd{g}", [128, WD_W], bf,
                                      kind="ExternalInput")
    out_t = nc.dram_tensor("out", [128, BL], f32, kind="ExternalOutput")

    with ExitStack() as ctx:
        tc = ctx.enter_context(tile.TileContext(nc))
        singles = ctx.enter_context(tc.tile_pool(name="singles", bufs=1))
        panels = ctx.enter_context(tc.tile_pool(name="panels", bufs=20))
        psum = ctx.enter_context(tc.tile_pool(name="psum", bufs=3, space="PSUM"))
        chain = ctx.enter_context(tc.tile_pool(name="chain", bufs=4))

        ident = singles.tile([BL, BL], f32)
        make_identity(nc, ident)
        xt = singles.tile([128, KX, BL], bf)
        nc.sync.dma_start(out=xt, in_=x_in[:])
        yall = singles.tile([128, NBLK * BL], bf)

        banks = {}     # g -> psum tile [64, cw]
        ptiles = {}    # (g, kt) -> (tile, local_kt)
        started = set()  # banks whose start=True matmul was emitted
        pending = {}   # g -> list of source kt indices not yet emitted
        alloc_hi = -1  # highest allocated group

        def pt(g, kt):
            t, lk = ptiles[(g, kt)]
            return t[:, lk, :]

        chunk_meta = {}  # g -> [(ci, k0, k1), ...] not yet DMA'd

        def alloc_group(g):
            banks[g] = psum.tile([64, _grp_cw(g)], f32, tag="bt", bufs=K_BT,
                                 name=f"bank{g}")
            pending[g] = list(range(KX)) + [
                KX + s for s in range(0, _grp_dmax(g) - 1)
            ]  # x tiles + Y sources 0..d_max-2

        wdt = {}  # g -> wd strip tile [128, 768]
        for g in range(NGRP):
            chunk_meta[g] = list(enumerate(_grp_chunks(g)))
            if _grp_full(g):
                chunk_meta[g].append(("wd", (KX + GROUP * g + 1, 0)))

        def pump_dma(d):
            """JIT panel loads, decoupled from bank allocation: a chunk's DMA
            is emitted ~K_MARGIN blocks before its sources become available
            (but no earlier than K_PF blocks before its group starts), so
            late groups' bulk streams early and the post-DMA tail is short."""
            for g in sorted(chunk_meta):
                rest = []
                for ci, (k0, k1) in chunk_meta[g]:
                    if d < max(k0 - KX - K_MARGIN, GROUP * g - K_PF):
                        rest.append((ci, (k0, k1)))
                    elif ci == "wd":
                        wtile = panels.tile([128, WD_W], bf, tag="wd", bufs=4,
                                            name=f"wd{g}")
                        nc.sync.dma_start(out=wtile, in_=wd_in[g][:])
                        wdt[g] = wtile
                    else:
                        cw = _grp_cw(g)
                        ptile = panels.tile(
                            [128, k1 - k0, cw], bf, tag=f"pan{cw}",
                            bufs=(20 if cw == 512 else 6),
                            name=f"p{g}_{ci}",
                        )
                        nc.sync.dma_start(out=ptile, in_=w_in[(g, ci)][:])
                        for kk in range(k0, k1):
                            ptiles[(g, kk)] = (ptile, kk - k0)
                if rest:
                    chunk_meta[g] = rest
                else:
                    del chunk_meta[g]

        def ldiag_ap(d):
            g, dc = d // GROUP, d % GROUP
            if not _grp_full(g) or dc == 0:
                return pt(g, KX + d)[:, ts(dc, 128)]
            return wdt[g][:, ds((0, 384, 640)[dc - 1], 128)]

        def wprev_ap(d):
            g, dc = d // GROUP, d % GROUP  # row KX+d-1, cols dc*128:+128
            if not _grp_full(g) or dc <= 1:
                return pt(g, KX + d - 1)[:, ts(dc, 128)]
            return wdt[g][:, ds((128, 512)[dc - 2], 128)]

        def emit_stream(g, kt):
            lhsT = xt[:, kt, :] if kt < KX else yall[:, ts(kt - KX, BL)]
            first = g not in started
            if first:
                started.add(g)
            last = kt == KX + _grp_dmax(g) - 2
            if _grp_full(g) and kt == KX + GROUP * g + 1:
                # trimmed last source: only its dest-block-3 columns exist
                nc.tensor.matmul(
                    banks[g][:, ds(384, 128)], lhsT=lhsT,
                    rhs=wdt[g][:, ds(256, 128)], start=first, stop=last,
                )
            else:
                nc.tensor.matmul(
                    banks[g], lhsT=lhsT, rhs=pt(g, kt), start=first, stop=last
                )

        def can_emit(kt, d):
            return kt < KX or kt - KX <= d - 1

        def flush(g, d):
            """Emit all pending source MMs for bank g allowed at iter d."""
            keep = []
            for kt in pending[g]:
                if can_emit(kt, d):
                    emit_stream(g, kt)
                else:
                    keep.append(kt)
            pending[g] = keep

        def drip(d, k):
            for g in sorted(pending):
                while pending[g] and k > 0:
                    kt = pending[g][0]
                    if not can_emit(kt, d):
                        break
                    pending[g].pop(0)
                    emit_stream(g, kt)
                    k -= 1

        def prework(d):
            """Copy+transpose block d's base slice, seed y0, queue MM1 dep."""
            g, dc = d // GROUP, d % GROUP
            flush(g, d - 1)  # slice d needs sources <= d-2 (emitted <= iter d-1)
            sb_bt = chain.tile([64, 128], f32, tag="sbt")
            nc.vector.tensor_copy(sb_bt, banks[g][:, ts(dc, 128)])
            wa = psum.tile([128, BL], f32, tag="wk", bufs=K_WK, name=f"wa{d}")
            wb = psum.tile([128, BL], f32, tag="wk", bufs=K_WK, name=f"wb{d}")
            nc.tensor.matmul(wa, lhsT=sb_bt, rhs=ident, is_transpose=True,
                             start=True, stop=False)
            nc.tensor.matmul(wb, lhsT=sb_bt, rhs=ident, is_transpose=True,
                             start=True, stop=False)
            y0 = chain.tile([128, BL], bf, tag="yc", name=f"y0_{d}")
            nc.scalar.activation(out=y0, in_=wa, func=Tanh)
            return wa, wb, y0

        state = {}  # d -> (wa, wb, y0)
        for g in range(min((LOOKAHEAD + GROUP - 1) // GROUP + 1, NGRP)):
            alloc_group(g)
            alloc_hi = g
        pump_dma(0)
        state[0] = prework(0)

        for d in range(NBLK):
            g, dc = d // GROUP, d % GROUP
            want = min((d + LOOKAHEAD) // GROUP, NGRP - 1)
            while alloc_hi < want:
                alloc_hi += 1
                alloc_group(alloc_hi)
            pump_dma(d)
            wa, wb, y0 = state.pop(d)
            ldiag = ldiag_ap(d)  # strictly-lower masked on host
            # ---- critical path: two decoupled chains ----
            # y1-chain: A_d += W_prev @ y1_{d-1} (stale) + L @ y0 -> y1 = tanh(A)
            # y2-chain: B_d additionally patches W_prev @ (y2-y1)_{d-1}, then
            #           += L @ y1 -> y2 = tanh(B). Base of B is exact.
            if d > 0:
                wprev = wprev_ap(d)
                nc.tensor.matmul(wa, lhsT=wprev, rhs=y1_prev, start=False,
                                 stop=False)
                nc.tensor.matmul(wb, lhsT=wprev, rhs=y1_prev, start=False,
                                 stop=False)
            nc.tensor.matmul(wa, lhsT=ldiag, rhs=y0, start=False, stop=True)
            y1 = chain.tile([128, BL], bf, tag="yc", name=f"y1_{d}")
            nc.scalar.activation(out=y1, in_=wa, func=Tanh)
            if d > 0:
                dlt = chain.tile([128, BL], bf, tag="dt", name=f"dt{d}")
                nc.vector.tensor_sub(dlt, yall[:, ts(d - 1, BL)], y1_prev)
                nc.tensor.matmul(wb, lhsT=wprev, rhs=dlt, start=False, stop=False)
            drip(d, K_DRIP1)
            nc.tensor.matmul(wb, lhsT=ldiag, rhs=y1, start=False, stop=True)
            y1_prev = y1
            if d < NBLK - 1:
                nc.scalar.activation(out=yall[:, ts(d, BL)], in_=wb, func=Tanh)
            else:
                yfin = chain.tile([128, BL], f32, tag="yf")
                nc.scalar.activation(out=yfin, in_=wb, func=Tanh)
                ofin = chain.tile([128, BL], f32, tag="of")
                nc.scalar.activation(out=ofin, in_=yfin, func=Sigmoid)
                nc.sync.dma_start(out=out_t[:], in_=ofin)
            # ---- pre-work for next block (overlaps this block's tail) ----
            if d + 1 < NBLK:
                state[d + 1] = prework(d + 1)
                drip(d, K_DRIP2)
    nc.compile()
    return nc


def _get_module():
    if "nc" not in _CACHE:
        _CACHE["nc"] = _build_module()
    return _CACHE["nc"]


_STRICT_LOWER = (np.arange(NB)[:, None] < np.arange(NB)[None, :]).astype(np.float32)


def _pack_w(W):
    """Group panels: pan[p, kt, c] = W[512*g + c, kt*128 + p], bf16, chunked.

    Each group's diagonal 128x128 sub-tiles are masked strictly-lower."""
    maps = {}
    W = np.asarray(W, np.float32)
    for g in range(NGRP):
        cw = _grp_cw(g)
        kt_n = _grp_kt(g)
        c0 = 512 * g
        blk = W[c0 : c0 + cw, : kt_n * 128]          # [c, kt*128]
        pan = np.ascontiguousarray(
            blk.reshape(cw, kt_n, 128).transpose(2, 1, 0)
        )                                             # [p, kt, c]
        for dc in range(cw // 128):
            d = GROUP * g + dc
            pan[:, KX + d, dc * 128 : (dc + 1) * 128] *= _STRICT_LOWER
        pan = pan.astype(BF16)
        for ci, (k0, k1) in enumerate(_grp_chunks(g)):
            maps[f"w{g}_{ci}"] = np.ascontiguousarray(pan[:, k0:k1, :])
        if _grp_full(g):
            r = KX + GROUP * g + 1
            maps[f"wd{g}"] = np.ascontiguousarray(
                np.concatenate(
                    [pan[:, r, 128:], pan[:, r + 1, 256:], pan[:, r + 2, 384:]],
                    axis=1,
                )
            )
    return maps


def _pack_x(xs):
    """xt[p, kt, c] = xs[c, kt*128 + p], bf16. xs: [BL, IN]."""
    return np.ascontiguousarray(
        np.asarray(xs, np.float32).reshape(BL, KX, 128).transpose(2, 1, 0)
    ).astype(BF16)


def kernel(x, W, output_size=OUT):
    from concourse.bass_utils import run_bass_kernel_spmd

    assert int(output_size) == OUT
    x = np.asarray(x, np.float32)
    assert x.shape == (B, IN) and np.asarray(W).shape == (NN, IN + NN)

    nc = _get_module()
    wmaps = _pack_w(W)
    in_maps = [
        {"xt": _pack_x(x[ci * BL : (ci + 1) * BL]), **wmaps} for ci in range(NCORES)
    ]
    res = run_bass_kernel_spmd(nc, in_maps, core_ids=list(range(NCORES)))
    out = np.empty((B, OUT), np.float32)
    for ci in range(NCORES):
        out[ci * BL : (ci + 1) * BL] = res.results[ci]["out"].T
    return out



# revision 5
# speedup vs baseline: 1.2215x; 1.2215x over previous
"""Trainium2 Bass kernel for nn_DAG_61246233641129 (gnn_message_passing).

Math: sequential DAG over N=4224 nodes, out_j = tanh(x @ W[j,:1024] +
sum_{i<j} out_i * W[j,1024+i]); final output = sigmoid of last 128 nodes'
outputs, shape [512, 128].

Strategy (hardcoded, self-contained):
  * Data-parallel: batch 512 sharded 8 ways (64 rows/core), W replicated.
    Only the needed lower-block-triangle of W is packed (bf16, ~26MB/core;
    mostly-unused top rows of each panel trimmed into compact strips) so
    HBM traffic is near the useful-bytes floor (~76us/core) -- the kernel
    is DMA-bound, cost-model end-to-end ~93us.
  * Nodes in 33 blocks of 128; 4 blocks share a [64 batch, 512 node] PSUM
    bank. Cross-block/input contributions are PE matmuls with the small
    x/Y tile stationary and W streaming 512-wide, dripped between
    critical-path ops; panel chunks are DMA'd just-in-time by source
    availability so the post-DMA dependent tail stays short.
  * Per block the bank slice is copied+PE-transposed into TWO
    [128 node, 64 batch] work banks A and B, solving the intra-block
    recurrence y = tanh(base + L_strict @ y) by seeded fixed point as two
    decoupled one-ACT chains: y0 = tanh(partial base) runs a block early;
    A += W_prev @ y1_prev (stale) + L @ y0 -> y1 = tanh(A);
    B additionally patches W_prev @ (y2 - y1)_prev, += L @ y1 ->
    y2 = tanh(B) (exact base). HW-verified vs the jax reference:
    max abs err 4.3e-3, rms rel 1.3e-3 (bf16 quantization floor).
"""

import numpy as np
import ml_dtypes

BF16 = ml_dtypes.bfloat16
F8 = ml_dtypes.float8_e3m4
WSCALE = 64.0  # weights stored as e3m4 * WSCALE; folded out via ACT scale

B = 512            # batch
IN = 1024          # input features
NN = 4224          # nodes
OUT = 128          # output nodes
NCORES = 8
BL = B // NCORES   # 64 batch rows per core
NB = 128           # node block
NBLK = NN // NB    # 33
KX = IN // 128     # 8 input k-tiles
GROUP = 4          # node blocks per [64, 512] PSUM bank
NGRP = (NBLK + GROUP - 1) // GROUP  # 9 (last group has 1 block)
CHUNK = 8          # k-tiles per DMA chunk of a panel (env K_CHUNK)
import os

LOOKAHEAD = int(os.environ.get("K_LOOKAHEAD", "8"))  # blocks of early group alloc
K_DRIP1 = int(os.environ.get("K_DRIP1", "1"))  # drip MMs inside the y1 window
K_DRIP2 = int(os.environ.get("K_DRIP2", "5"))  # drip MMs at end of block
K_MARGIN = int(os.environ.get("K_MARGIN", "3"))  # chunk DMA prefetch margin
K_PF = int(os.environ.get("K_PF", "10"))  # max blocks of early chunk DMA
K_BT = int(os.environ.get("K_BT", "4"))   # bt-bank buffers
K_WK = int(os.environ.get("K_WK", "4"))   # work-bank buffers
CHUNK = int(os.environ.get("K_CHUNK", str(CHUNK)))

_CACHE = {}


def _grp_cw(g):
    return 128 * min(GROUP, NBLK - GROUP * g)


def _grp_dmax(g):
    return min(GROUP * g + GROUP - 1, NBLK - 1)


def _grp_kt(g):
    return KX + _grp_dmax(g) + 1


def _grp_full(g):
    return _grp_cw(g) == 512


def _grp_ktm(g):
    """Main-panel rows: full groups push their last 3 (mostly unused) rows
    into a compact 'wd' strip; the last narrow group keeps everything."""
    return KX + GROUP * g + 1 if _grp_full(g) else _grp_kt(g)


def _grp_chunks(g):
    kt_n = _grp_ktm(g)
    return [(c0, min(c0 + CHUNK, kt_n)) for c0 in range(0, kt_n, CHUNK)]


# wd strip layout (full groups): [row KX+4g+1 cols 128:512 | row KX+4g+2
# cols 256:512 | row KX+4g+3 cols 384:512] -> local offsets 0/384/640, 768 wide
WD_W = 768


def _build_module():
    import concourse.mybir as mybir
    import concourse.tile as tile
    from concourse import bacc
    from concourse.bass import ds, ts
    from concourse.masks import make_identity
    from contextlib import ExitStack

    bf = mybir.dt.bfloat16
    f8 = mybir.dt.float8e3
    f32 = mybir.dt.float32
    Tanh = mybir.ActivationFunctionType.Tanh
    Sigmoid = mybir.ActivationFunctionType.Sigmoid
    INV = 1.0 / WSCALE

    nc = bacc.Bacc()
    x_in = nc.dram_tensor("xt", [128, KX, BL], bf, kind="ExternalInput")
    w_in = {}
    wd_in = {}
    for g in range(NGRP):
        cw = _grp_cw(g)
        for ci, (k0, k1) in enumerate(_grp_chunks(g)):
            w_in[(g, ci)] = nc.dram_tensor(
                f"w{g}_{ci}", [128, k1 - k0, cw], f8, kind="ExternalInput"
            )
        if _grp_full(g):
            wd_in[g] = nc.dram_tensor(f"wd{g}", [128, WD_W], f8,
                                      kind="ExternalInput")
    out_t = nc.dram_tensor("out", [128, BL], f32, kind="ExternalOutput")

    with ExitStack() as ctx:
        tc = ctx.enter_context(tile.TileContext(nc))
        singles = ctx.enter_context(tc.tile_pool(name="singles", bufs=1))
        panels = ctx.enter_context(tc.tile_pool(name="panels", bufs=20))
        psum = ctx.enter_context(tc.tile_pool(name="psum", bufs=3, space="PSUM"))
        chain = ctx.enter_context(tc.tile_pool(name="chain", bufs=4))

        ident = singles.tile([BL, BL], f32)
        make_identity(nc, ident)
        xt = singles.tile([128, KX, BL], bf)
        nc.sync.dma_start(out=xt, in_=x_in[:])
        yall = singles.tile([128, NBLK * BL], bf)

        banks = {}     # g -> psum tile [64, cw]
        ptiles = {}    # (g, kt) -> (tile, local_kt)
        started = set()  # banks whose start=True matmul was emitted
        pending = {}   # g -> list of source kt indices not yet emitted
        alloc_hi = -1  # highest allocated group

        def pt(g, kt):
            t, lk = ptiles[(g, kt)]
            return t[:, lk, :]

        chunk_meta = {}  # g -> [(ci, k0, k1), ...] not yet DMA'd

        def alloc_group(g):
            banks[g] = psum.tile([64, _grp_cw(g)], f32, tag="bt", bufs=K_BT,
                                 name=f"bank{g}")
            pending[g] = list(range(KX)) + [
                KX + s for s in range(0, _grp_dmax(g) - 1)
            ]  # x tiles + Y sources 0..d_max-2

        wdt = {}  # g -> wd strip tile [128, 768]
        for g in range(NGRP):
            chunk_meta[g] = list(enumerate(_grp_chunks(g)))
            if _grp_full(g):
                chunk_meta[g].append(("wd", (KX + GROUP * g + 1, 0)))

        def pump_dma(d):
            """JIT panel loads, decoupled from bank allocation: a chunk's DMA
            is emitted ~K_MARGIN blocks before its sources become available
            (but no earlier than K_PF blocks before its group starts), so
            late groups' bulk streams early and the post-DMA tail is short."""
            for g in sorted(chunk_meta):
                rest = []
                for ci, (k0, k1) in chunk_meta[g]:
                    if d < max(k0 - KX - K_MARGIN, GROUP * g - K_PF):
                        rest.append((ci, (k0, k1)))
                    elif ci == "wd":
                        wtile = panels.tile([128, WD_W], f8, tag="wd", bufs=4,
                                            name=f"wd{g}")
                        nc.sync.dma_start(out=wtile, in_=wd_in[g][:])
                        wdt[g] = wtile
                    else:
                        cw = _grp_cw(g)
                        ptile = panels.tile(
                            [128, k1 - k0, cw], f8, tag=f"pan{cw}",
                            bufs=(20 if cw == 512 else 6),
                            name=f"p{g}_{ci}",
                        )
                        nc.sync.dma_start(out=ptile, in_=w_in[(g, ci)][:])
                        for kk in range(k0, k1):
                            ptiles[(g, kk)] = (ptile, kk - k0)
                if rest:
                    chunk_meta[g] = rest
                else:
                    del chunk_meta[g]

        def ldiag_ap(d):
            g, dc = d // GROUP, d % GROUP
            if not _grp_full(g) or dc == 0:
                return pt(g, KX + d)[:, ts(dc, 128)]
            return wdt[g][:, ds((0, 384, 640)[dc - 1], 128)]

        def wprev_ap(d):
            g, dc = d // GROUP, d % GROUP  # row KX+d-1, cols dc*128:+128
            if not _grp_full(g) or dc <= 1:
                return pt(g, KX + d - 1)[:, ts(dc, 128)]
            return wdt[g][:, ds((128, 512)[dc - 2], 128)]

        def emit_stream(g, kt):
            lhsT = xt[:, kt, :] if kt < KX else yall[:, ts(kt - KX, BL)]
            first = g not in started
            if first:
                started.add(g)
            last = kt == KX + _grp_dmax(g) - 2
            if _grp_full(g) and kt == KX + GROUP * g + 1:
                # trimmed last source: only its dest-block-3 columns exist
                nc.tensor.matmul(
                    banks[g][:, ds(384, 128)], lhsT=lhsT,
                    rhs=wdt[g][:, ds(256, 128)], start=first, stop=last,
                )
            else:
                nc.tensor.matmul(
                    banks[g], lhsT=lhsT, rhs=pt(g, kt), start=first, stop=last
                )

        def can_emit(kt, d):
            return kt < KX or kt - KX <= d - 1

        def flush(g, d):
            """Emit all pending source MMs for bank g allowed at iter d."""
            keep = []
            for kt in pending[g]:
                if can_emit(kt, d):
                    emit_stream(g, kt)
                else:
                    keep.append(kt)
            pending[g] = keep

        def drip(d, k):
            for g in sorted(pending):
                while pending[g] and k > 0:
                    kt = pending[g][0]
                    if not can_emit(kt, d):
                        break
                    pending[g].pop(0)
                    emit_stream(g, kt)
                    k -= 1

        def prework(d):
            """Copy+transpose block d's base slice, seed y0, queue MM1 dep."""
            g, dc = d // GROUP, d % GROUP
            flush(g, d - 1)  # slice d needs sources <= d-2 (emitted <= iter d-1)
            sb_bt = chain.tile([64, 128], f32, tag="sbt")
            nc.vector.tensor_copy(sb_bt, banks[g][:, ts(dc, 128)])
            wa = psum.tile([128, BL], f32, tag="wk", bufs=K_WK, name=f"wa{d}")
            wb = psum.tile([128, BL], f32, tag="wk", bufs=K_WK, name=f"wb{d}")
            nc.tensor.matmul(wa, lhsT=sb_bt, rhs=ident, is_transpose=True,
                             start=True, stop=False)
            nc.tensor.matmul(wb, lhsT=sb_bt, rhs=ident, is_transpose=True,
                             start=True, stop=False)
            y0 = chain.tile([128, BL], bf, tag="yc", name=f"y0_{d}")
            nc.scalar.activation(out=y0, in_=wa, func=Tanh, scale=INV)
            return wa, wb, y0

        state = {}  # d -> (wa, wb, y0)
        for g in range(min((LOOKAHEAD + GROUP - 1) // GROUP + 1, NGRP)):
            alloc_group(g)
            alloc_hi = g
        pump_dma(0)
        state[0] = prework(0)

        for d in range(NBLK):
            g, dc = d // GROUP, d % GROUP
            want = min((d + LOOKAHEAD) // GROUP, NGRP - 1)
            while alloc_hi < want:
                alloc_hi += 1
                alloc_group(alloc_hi)
            pump_dma(d)
            wa, wb, y0 = state.pop(d)
            ldiag = ldiag_ap(d)  # strictly-lower masked on host
            # ---- critical path: two decoupled chains ----
            # y1-chain: A_d += W_prev @ y1_{d-1} (stale) + L @ y0 -> y1 = tanh(A)
            # y2-chain: B_d additionally patches W_prev @ (y2-y1)_{d-1}, then
            #           += L @ y1 -> y2 = tanh(B). Base of B is exact.
            if d > 0:
                wprev = wprev_ap(d)
                nc.tensor.matmul(wa, lhsT=wprev, rhs=y1_prev, start=False,
                                 stop=False)
                nc.tensor.matmul(wb, lhsT=wprev, rhs=y1_prev, start=False,
                                 stop=False)
            nc.tensor.matmul(wa, lhsT=ldiag, rhs=y0, start=False, stop=True)
            y1 = chain.tile([128, BL], bf, tag="yc", name=f"y1_{d}")
            nc.scalar.activation(out=y1, in_=wa, func=Tanh, scale=INV)
            if d > 0:
                dlt = chain.tile([128, BL], bf, tag="dt", name=f"dt{d}")
                nc.vector.tensor_sub(dlt, yall[:, ts(d - 1, BL)], y1_prev)
                nc.tensor.matmul(wb, lhsT=wprev, rhs=dlt, start=False, stop=False)
            drip(d, K_DRIP1)
            nc.tensor.matmul(wb, lhsT=ldiag, rhs=y1, start=False, stop=True)
            y1_prev = y1
            if d < NBLK - 1:
                nc.scalar.activation(out=yall[:, ts(d, BL)], in_=wb, func=Tanh, scale=INV)
            else:
                yfin = chain.tile([128, BL], f32, tag="yf")
                nc.scalar.activation(out=yfin, in_=wb, func=Tanh, scale=INV)
                ofin = chain.tile([128, BL], f32, tag="of")
                nc.scalar.activation(out=ofin, in_=yfin, func=Sigmoid)
                nc.sync.dma_start(out=out_t[:], in_=ofin)
            # ---- pre-work for next block (overlaps this block's tail) ----
            if d + 1 < NBLK:
                state[d + 1] = prework(d + 1)
                drip(d, K_DRIP2)
    nc.compile()
    return nc


def _get_module():
    if "nc" not in _CACHE:
        _CACHE["nc"] = _build_module()
    return _CACHE["nc"]


_STRICT_LOWER = (np.arange(NB)[:, None] < np.arange(NB)[None, :]).astype(np.float32)


def _pack_w(W):
    """Group panels: pan[p, kt, c] = W[512*g + c, kt*128 + p], bf16, chunked.

    Each group's diagonal 128x128 sub-tiles are masked strictly-lower."""
    maps = {}
    W = np.asarray(W, np.float32)
    for g in range(NGRP):
        cw = _grp_cw(g)
        kt_n = _grp_kt(g)
        c0 = 512 * g
        blk = W[c0 : c0 + cw, : kt_n * 128]          # [c, kt*128]
        pan = np.ascontiguousarray(
            blk.reshape(cw, kt_n, 128).transpose(2, 1, 0)
        )                                             # [p, kt, c]
        for dc in range(cw // 128):
            d = GROUP * g + dc
            pan[:, KX + d, dc * 128 : (dc + 1) * 128] *= _STRICT_LOWER
        pan = (pan * WSCALE).astype(F8)
        for ci, (k0, k1) in enumerate(_grp_chunks(g)):
            maps[f"w{g}_{ci}"] = np.ascontiguousarray(pan[:, k0:k1, :])
        if _grp_full(g):
            r = KX + GROUP * g + 1
            maps[f"wd{g}"] = np.ascontiguousarray(
                np.concatenate(
                    [pan[:, r, 128:], pan[:, r + 1, 256:], pan[:, r + 2, 384:]],
                    axis=1,
                )
            )
    return maps


def _pack_x(xs):
    """xt[p, kt, c] = xs[c, kt*128 + p], bf16. xs: [BL, IN]."""
    return np.ascontiguousarray(
        np.asarray(xs, np.float32).reshape(BL, KX, 128).transpose(2, 1, 0)
    ).astype(BF16)


def kernel(x, W, output_size=OUT):
    from concourse.bass_utils import run_bass_kernel_spmd

    assert int(output_size) == OUT
    x = np.asarray(x, np.float32)
    assert x.shape == (B, IN) and np.asarray(W).shape == (NN, IN + NN)

    nc = _get_module()
    wmaps = _pack_w(W)
    in_maps = [
        {"xt": _pack_x(x[ci * BL : (ci + 1) * BL]), **wmaps} for ci in range(NCORES)
    ]
    res = run_bass_kernel_spmd(nc, in_maps, core_ids=list(range(NCORES)))
    out = np.empty((B, OUT), np.float32)
    for ci in range(NCORES):
        out[ci * BL : (ci + 1) * BL] = res.results[ci]["out"].T
    return out



# revision 6
# speedup vs baseline: 1.4228x; 1.1647x over previous
"""Trainium2 Bass kernel for nn_DAG_61246233641129 (gnn_message_passing).

Math: sequential DAG over N=4224 nodes, out_j = tanh(x @ W[j,:1024] +
sum_{i<j} out_i * W[j,1024+i]); final output = sigmoid of last 128 nodes'
outputs, shape [512, 128].

Strategy (hardcoded, self-contained):
  * Data-parallel: batch 512 sharded 8 ways (64 rows/core), W replicated.
    Only the needed lower-block-triangle of W is packed, quantized to
    fp8-e4m3 (x WSCALE, folded back out via the ACT scale input), so HBM
    traffic is ~13.7MB/core (~38us at 360 B/ns). x and the published node
    outputs y2 are also fp8-e4m3 so cross-block contributions run as
    DoubleRow matmuls (0.5 cycles/row): PE stream time ~21us.
  * Nodes in 33 blocks of 128; 4 blocks share a [64 batch, 512 node] PSUM
    bank. Cross-block/input sources stream as even-aligned fp8 DoubleRow
    PAIRS (panel layout is kt-major so a pair is two adjacent rows of one
    chunk tile); the pair covering the group's copy frontier is split into
    singles on demand. Chunks are DMA'd just-in-time by source
    availability; drip MMs pace leftover stream work between
    critical-path ops.
  * Per block the bank slice is copied+PE-transposed into TWO
    [128 node, 64 batch] work banks A and B, solving the intra-block
    recurrence y = tanh(base + L_strict @ y) by seeded fixed point as two
    decoupled one-ACT chains: y0 = tanh(partial base) runs a block early;
    A += W_prev @ y1_prev (stale) + L @ y0 -> y1 = tanh(A);
    B additionally patches W_prev @ (y2 - y1)_prev, += L @ y1 ->
    y2 = tanh(B) (exact base). y0/y1 bf16; y2 published fp8.
    Numpy-simulated end-to-end error with this exact dataflow: 1.58e-2
    (fp8 quantization floor), previously verified bit-matching on device.
"""

import numpy as np
import ml_dtypes

BF16 = ml_dtypes.bfloat16
F8E4 = ml_dtypes.float8_e4m3
WSCALE = 64.0  # weights stored as e4m3 * WSCALE; folded out via ACT scale

B = 512            # batch
IN = 1024          # input features
NN = 4224          # nodes
OUT = 128          # output nodes
NCORES = 8
BL = B // NCORES   # 64 batch rows per core
NB = 128           # node block
NBLK = NN // NB    # 33
KX = IN // 128     # 8 input k-tiles
GROUP = 4          # node blocks per [64, 512] PSUM bank
NGRP = (NBLK + GROUP - 1) // GROUP  # 9 (last group has 1 block)
CHUNK = 8          # k-tiles per DMA chunk of a panel (env K_CHUNK)
import os

LOOKAHEAD = int(os.environ.get("K_LOOKAHEAD", "8"))  # blocks of early group alloc
K_DRIP1 = int(os.environ.get("K_DRIP1", "1"))  # drip MMs inside the y1 window
K_DRIP2 = int(os.environ.get("K_DRIP2", "5"))  # drip MMs at end of block
K_MARGIN = int(os.environ.get("K_MARGIN", "3"))  # chunk DMA prefetch margin
K_PF = int(os.environ.get("K_PF", "10"))  # max blocks of early chunk DMA
K_BT = int(os.environ.get("K_BT", "4"))   # bt-bank buffers
K_WK = int(os.environ.get("K_WK", "4"))   # work-bank buffers
CHUNK = int(os.environ.get("K_CHUNK", str(CHUNK)))

_CACHE = {}


def _grp_cw(g):
    return 128 * min(GROUP, NBLK - GROUP * g)


def _grp_dmax(g):
    return min(GROUP * g + GROUP - 1, NBLK - 1)


def _grp_kt(g):
    return KX + _grp_dmax(g) + 1


def _grp_full(g):
    return _grp_cw(g) == 512


def _grp_ktm(g):
    """Main-panel rows: full groups push their last 3 (mostly unused) rows
    into a compact 'wd' strip; the last narrow group keeps everything."""
    return KX + GROUP * g + 1 if _grp_full(g) else _grp_kt(g)


def _grp_chunks(g):
    kt_n = _grp_ktm(g)
    return [(c0, min(c0 + CHUNK, kt_n)) for c0 in range(0, kt_n, CHUNK)]


def _grp_units(g):
    """Stream-emission units: ('p', kt0) = DoubleRow pair (kt0, kt0+1),
    ('s', kt) = single, ('t', kt) = trimmed single (wd strip).
    Pairs are even-aligned so they never straddle an 8-row chunk."""
    units = [("p", 2 * i) for i in range(KX // 2)]
    if _grp_full(g):
        units += [("p", KX + 2 * i) for i in range(2 * g)]   # Y s=0..4g-1
        units += [("s", KX + 4 * g), ("t", KX + 4 * g + 1)]
    else:
        smax = _grp_dmax(g) - 2
        units += [("p", KX + 2 * i) for i in range(smax // 2)]
        units += [("s", KX + smax)]
    return units


# wd strip layout (full groups): [row KX+4g+1 cols 128:512 | row KX+4g+2
# cols 256:512 | row KX+4g+3 cols 384:512] -> local offsets 0/384/640, 768 wide
WD_W = 768


def _build_module():
    import concourse.mybir as mybir
    import concourse.tile as tile
    from concourse import bacc
    from concourse.bass import ds, ts
    from concourse.masks import make_identity
    from contextlib import ExitStack

    bf = mybir.dt.bfloat16
    f8 = mybir.dt.float8e4
    f32 = mybir.dt.float32
    Tanh = mybir.ActivationFunctionType.Tanh
    Sigmoid = mybir.ActivationFunctionType.Sigmoid
    DR = mybir.MatmulPerfMode.DoubleRow
    INV = 1.0 / WSCALE

    nc = bacc.Bacc()
    x_in = nc.dram_tensor("xt", [128, KX, BL], f8, kind="ExternalInput")
    w_in = {}
    wd_in = {}
    for g in range(NGRP):
        cw = _grp_cw(g)
        for ci, (k0, k1) in enumerate(_grp_chunks(g)):
            w_in[(g, ci)] = nc.dram_tensor(
                f"w{g}_{ci}", [128, k1 - k0, cw], f8, kind="ExternalInput"
            )
        if _grp_full(g):
            wd_in[g] = nc.dram_tensor(f"wd{g}", [128, WD_W], f8,
                                      kind="ExternalInput")
    out_t = nc.dram_tensor("out", [128, BL], f32, kind="ExternalOutput")

    with ExitStack() as ctx:
        tc = ctx.enter_context(tile.TileContext(nc))
        singles = ctx.enter_context(tc.tile_pool(name="singles", bufs=1))
        panels = ctx.enter_context(tc.tile_pool(name="panels", bufs=20))
        psum = ctx.enter_context(tc.tile_pool(name="psum", bufs=3, space="PSUM"))
        chain = ctx.enter_context(tc.tile_pool(name="chain", bufs=4))

        ident = singles.tile([BL, BL], f32)
        make_identity(nc, ident)
        xt = singles.tile([128, KX, BL], f8)
        nc.sync.dma_start(out=xt, in_=x_in[:])
        yall = singles.tile([128, NBLK, BL], f8)

        banks = {}     # g -> psum tile [64, cw]
        ptiles = {}    # (g, kt) -> (tile, local_kt)
        started = set()  # banks whose start=True matmul was emitted
        pending = {}   # g -> list of emission units not yet emitted
        alloc_hi = -1  # highest allocated group

        def pt(g, kt):
            t, lk = ptiles[(g, kt)]
            return t[:, lk, :]

        chunk_meta = {}  # g -> [(ci, k0, k1), ...] not yet DMA'd

        def alloc_group(g):
            banks[g] = psum.tile([64, _grp_cw(g)], f32, tag="bt", bufs=K_BT,
                                 name=f"bank{g}")
            pending[g] = _grp_units(g)

        wdt = {}  # g -> wd strip tile [128, 768]
        for g in range(NGRP):
            chunk_meta[g] = list(enumerate(_grp_chunks(g)))
            if _grp_full(g):
                chunk_meta[g].append(("wd", (KX + GROUP * g + 1, 0)))

        def pump_dma(d):
            """JIT panel loads, decoupled from bank allocation: a chunk's DMA
            is emitted ~K_MARGIN blocks before its sources become available
            (but no earlier than K_PF blocks before its group starts), so
            late groups' bulk streams early and the post-DMA tail is short."""
            for g in sorted(chunk_meta):
                rest = []
                for ci, (k0, k1) in chunk_meta[g]:
                    if d < max(k0 - KX - K_MARGIN, GROUP * g - K_PF):
                        rest.append((ci, (k0, k1)))
                    elif ci == "wd":
                        wtile = panels.tile([128, WD_W], f8, tag="wd", bufs=4,
                                            name=f"wd{g}")
                        nc.sync.dma_start(out=wtile, in_=wd_in[g][:])
                        wdt[g] = wtile
                    else:
                        cw = _grp_cw(g)
                        ptile = panels.tile(
                            [128, k1 - k0, cw], f8, tag=f"pan{cw}",
                            bufs=(20 if cw == 512 else 6),
                            name=f"p{g}_{ci}",
                        )
                        nc.sync.dma_start(out=ptile, in_=w_in[(g, ci)][:])
                        for kk in range(k0, k1):
                            ptiles[(g, kk)] = (ptile, kk - k0)
                if rest:
                    chunk_meta[g] = rest
                else:
                    del chunk_meta[g]

        def ldiag_ap(d):
            g, dc = d // GROUP, d % GROUP
            if not _grp_full(g) or dc == 0:
                return pt(g, KX + d)[:, ts(dc, 128)]
            return wdt[g][:, ds((0, 384, 640)[dc - 1], 128)]

        def wprev_ap(d):
            g, dc = d // GROUP, d % GROUP  # row KX+d-1, cols dc*128:+128
            if not _grp_full(g) or dc <= 1:
                return pt(g, KX + d - 1)[:, ts(dc, 128)]
            return wdt[g][:, ds((128, 512)[dc - 2], 128)]

        def emit_stream(g, u):
            kind, kt = u
            first = g not in started
            if first:
                started.add(g)
            last_kt = KX + _grp_dmax(g) - 2
            if kind == "t":
                # trimmed last source: only its dest-block-3 columns exist
                nc.tensor.matmul(
                    banks[g][:, ds(384, 128)], lhsT=yall[:, kt - KX, :],
                    rhs=wdt[g][:, ds(256, 128)], start=first,
                    stop=kt == last_kt,
                )
            elif kind == "s":
                lhsT = xt[:, kt, :] if kt < KX else yall[:, kt - KX, :]
                nc.tensor.matmul(
                    banks[g], lhsT=lhsT, rhs=pt(g, kt), start=first,
                    stop=kt == last_kt,
                )
            else:  # DoubleRow pair (kt, kt+1); never the stop unit
                lhsT = (xt[:, ds(kt, 2), :] if kt < KX
                        else yall[:, ds(kt - KX, 2), :])
                t, lk = ptiles[(g, kt)]
                t2, lk2 = ptiles[(g, kt + 1)]
                assert t is t2 and lk2 == lk + 1
                nc.tensor.matmul(
                    banks[g], lhsT=lhsT, rhs=t[:, ds(lk, 2), :],
                    start=first, stop=False, perf_mode=DR,
                )

        def can_emit(u, d):
            kind, kt0 = u
            kt_last = kt0 + 1 if kind == "p" else kt0
            return kt_last < KX or kt_last - KX <= d - 1

        def flush(g, d):
            """Emit all pending source MMs for bank g allowed at iter d,
            splitting a pair whose second half is past the frontier."""
            keep = []
            for u in pending[g]:
                if can_emit(u, d):
                    emit_stream(g, u)
                elif u[0] == "p" and u[1] - KX <= d - 1:
                    emit_stream(g, ("s", u[1]))
                    keep.append(("s", u[1] + 1))
                else:
                    keep.append(u)
            pending[g] = keep

        def drip(d, k):
            for g in sorted(pending):
                while pending[g] and k > 0:
                    u = pending[g][0]
                    if not can_emit(u, d):
                        break
                    pending[g].pop(0)
                    emit_stream(g, u)
                    k -= 1

        def prework(d):
            """Copy+transpose block d's base slice, seed y0, queue MM1 dep."""
            g, dc = d // GROUP, d % GROUP
            flush(g, d - 1)  # slice d needs sources <= d-2 (emitted <= iter d-1)
            sb_bt = chain.tile([64, 128], f32, tag="sbt")
            nc.vector.tensor_copy(sb_bt, banks[g][:, ts(dc, 128)])
            wa = psum.tile([128, BL], f32, tag="wk", bufs=K_WK, name=f"wa{d}")
            wb = psum.tile([128, BL], f32, tag="wk", bufs=K_WK, name=f"wb{d}")
            nc.tensor.matmul(wa, lhsT=sb_bt, rhs=ident, is_transpose=True,
                             start=True, stop=False)
            nc.tensor.matmul(wb, lhsT=sb_bt, rhs=ident, is_transpose=True,
                             start=True, stop=False)
            y0 = chain.tile([128, BL], bf, tag="yc", name=f"y0_{d}")
            nc.scalar.activation(out=y0, in_=wa, func=Tanh, scale=INV)
            return wa, wb, y0

        state = {}  # d -> (wa, wb, y0)
        for g in range(min((LOOKAHEAD + GROUP - 1) // GROUP + 1, NGRP)):
            alloc_group(g)
            alloc_hi = g
        pump_dma(0)
        state[0] = prework(0)

        for d in range(NBLK):
            g, dc = d // GROUP, d % GROUP
            want = min((d + LOOKAHEAD) // GROUP, NGRP - 1)
            while alloc_hi < want:
                alloc_hi += 1
                alloc_group(alloc_hi)
            pump_dma(d)
            wa, wb, y0 = state.pop(d)
            ldiag = ldiag_ap(d)  # strictly-lower masked on host
            # ---- critical path: two decoupled chains ----
            # y1-chain: A_d += W_prev @ y1_{d-1} (stale) + L @ y0 -> y1 = tanh(A)
            # y2-chain: B_d additionally patches W_prev @ (y2-y1)_{d-1}, then
            #           += L @ y1 -> y2 = tanh(B). Base of B is exact.
            if d > 0:
                wprev = wprev_ap(d)
                nc.tensor.matmul(wa, lhsT=wprev, rhs=y1_prev, start=False,
                                 stop=False)
                nc.tensor.matmul(wb, lhsT=wprev, rhs=y1_prev, start=False,
                                 stop=False)
            nc.tensor.matmul(wa, lhsT=ldiag, rhs=y0, start=False, stop=True)
            y1 = chain.tile([128, BL], bf, tag="yc", name=f"y1_{d}")
            nc.scalar.activation(out=y1, in_=wa, func=Tanh, scale=INV)
            if d > 0:
                dlt = chain.tile([128, BL], bf, tag="dt", name=f"dt{d}")
                nc.vector.tensor_sub(dlt, yall[:, d - 1, :], y1_prev)
                nc.tensor.matmul(wb, lhsT=wprev, rhs=dlt, start=False, stop=False)
            drip(d, K_DRIP1)
            nc.tensor.matmul(wb, lhsT=ldiag, rhs=y1, start=False, stop=True)
            y1_prev = y1
            if d < NBLK - 1:
                nc.scalar.activation(out=yall[:, d, :], in_=wb, func=Tanh,
                                     scale=INV)
            else:
                yfin = chain.tile([128, BL], f32, tag="yf")
                nc.scalar.activation(out=yfin, in_=wb, func=Tanh, scale=INV)
                ofin = chain.tile([128, BL], f32, tag="of")
                nc.scalar.activation(out=ofin, in_=yfin, func=Sigmoid)
                nc.sync.dma_start(out=out_t[:], in_=ofin)
            # ---- pre-work for next block (overlaps this block's tail) ----
            if d + 1 < NBLK:
                state[d + 1] = prework(d + 1)
                drip(d, K_DRIP2)
    nc.compile()
    return nc


def _get_module():
    if "nc" not in _CACHE:
        _CACHE["nc"] = _build_module()
    return _CACHE["nc"]


_STRICT_LOWER = (np.arange(NB)[:, None] < np.arange(NB)[None, :]).astype(np.float32)


def _pack_w(W):
    """Group panels: pan[p, kt, c] = W[512*g + c, kt*128 + p], e4m3*WSCALE,
    chunked. Each group's diagonal 128x128 sub-tiles are masked
    strictly-lower before quantization."""
    maps = {}
    W = np.asarray(W, np.float32)
    for g in range(NGRP):
        cw = _grp_cw(g)
        kt_n = _grp_kt(g)
        c0 = 512 * g
        blk = W[c0 : c0 + cw, : kt_n * 128]          # [c, kt*128]
        pan = np.ascontiguousarray(
            blk.reshape(cw, kt_n, 128).transpose(2, 1, 0)
        )                                             # [p, kt, c]
        for dc in range(cw // 128):
            d = GROUP * g + dc
            pan[:, KX + d, dc * 128 : (dc + 1) * 128] *= _STRICT_LOWER
        pan = (pan * WSCALE).astype(F8E4)
        for ci, (k0, k1) in enumerate(_grp_chunks(g)):
            maps[f"w{g}_{ci}"] = np.ascontiguousarray(pan[:, k0:k1, :])
        if _grp_full(g):
            r = KX + GROUP * g + 1
            maps[f"wd{g}"] = np.ascontiguousarray(
                np.concatenate(
                    [pan[:, r, 128:], pan[:, r + 1, 256:], pan[:, r + 2, 384:]],
                    axis=1,
                )
            )
    return maps


def _pack_x(xs):
    """xt[p, kt, c] = xs[c, kt*128 + p], e4m3 (unit scale). xs: [BL, IN]."""
    return np.ascontiguousarray(
        np.asarray(xs, np.float32).reshape(BL, KX, 128).transpose(2, 1, 0)
    ).astype(F8E4)


def kernel(x, W, output_size=OUT):
    from concourse.bass_utils import run_bass_kernel_spmd

    assert int(output_size) == OUT
    x = np.asarray(x, np.float32)
    assert x.shape == (B, IN) and np.asarray(W).shape == (NN, IN + NN)

    nc = _get_module()
    wmaps = _pack_w(W)
    in_maps = [
        {"xt": _pack_x(x[ci * BL : (ci + 1) * BL]), **wmaps} for ci in range(NCORES)
    ]
    res = run_bass_kernel_spmd(nc, in_maps, core_ids=list(range(NCORES)))
    out = np.empty((B, OUT), np.float32)
    for ci in range(NCORES):
        out[ci * BL : (ci + 1) * BL] = res.results[ci]["out"].T
    return out


# revision 27
# speedup vs baseline: 1.7021x; 1.1964x over previous
"""Trainium2 Bass kernel for nn_DAG_61246233641129 (gnn_message_passing).

Math: sequential DAG over N=4224 nodes, out_j = tanh(x @ W[j,:1024] +
sum_{i<j} out_i * W[j,1024+i]); final output = sigmoid of last 128 nodes'
outputs, shape [512, 128].

Strategy (hardcoded, self-contained):
  * Data-parallel: batch 512 sharded 8 ways (64 rows/core), W replicated.
    Only the needed lower-block-triangle of W is packed, quantized to
    fp8-e4m3 (x WSCALE, folded back out via the ACT scale input), so HBM
    traffic is ~13.7MB/core (~38us at 360 B/ns). x and the published node
    outputs y2 are also fp8-e4m3 so cross-block contributions run as
    DoubleRow matmuls (0.5 cycles/row): PE stream time ~21us.
  * Nodes in 33 blocks of 128; 4 blocks share a [64 batch, 512 node] PSUM
    bank. Cross-block/input sources stream as even-aligned fp8 DoubleRow
    PAIRS (panel layout is kt-major so a pair is two adjacent rows of one
    chunk tile); the pair covering the group's copy frontier is split into
    singles on demand. Chunks are DMA'd just-in-time by source
    availability; drip MMs pace leftover stream work between
    critical-path ops.
  * Per block the bank slice is copied+PE-transposed into TWO
    [128 node, 64 batch] work banks A and B, solving the intra-block
    recurrence y = tanh(base + L_strict @ y) as two decoupled one-ACT
    chains (2 ACTs/block on the serial path):
    A += W_prev @ y1_prev (stale) -> y1 = tanh(A);
    B += W_prev @ y2_prev (exact) + L @ y1 -> y2 = tanh(B).
    y1 bf16 (chain-only); y2 published fp8 for the DoubleRow streams.
    Final sigmoid via sigmoid(y) = 0.5*tanh(y/2)+0.5 (no ACT-table swap).
    Numpy-simulated end-to-end error with this exact dataflow: 1.63e-2
    (fp8 + one-sweep fixed point), sim previously verified bit-matching
    on device.
"""

import numpy as np
import ml_dtypes

BF16 = ml_dtypes.bfloat16
F8E4 = ml_dtypes.float8_e4m3
WSCALE = 64.0  # weights stored as e4m3 * WSCALE; folded out via ACT scale

B = 512            # batch
IN = 1024          # input features
NN = 4224          # nodes
OUT = 128          # output nodes
NCORES = 8
BL = B // NCORES   # 64 batch rows per core
NB = 128           # node block
NBLK = NN // NB    # 33
KX = IN // 128     # 8 input k-tiles
GROUP = 4          # node blocks per [64, 512] PSUM bank
NGRP = (NBLK + GROUP - 1) // GROUP  # 9 (last group has 1 block)
CHUNK = 8          # k-tiles per DMA chunk of a panel (env K_CHUNK)
import os

LOOKAHEAD = int(os.environ.get("K_LOOKAHEAD", "8"))  # blocks of early group alloc
K_DRIP1 = int(os.environ.get("K_DRIP1", "3"))  # drip MMs inside the y1 window
K_DRIP2 = int(os.environ.get("K_DRIP2", "9"))  # drip MMs at end of block
K_MARGIN = int(os.environ.get("K_MARGIN", "3"))  # chunk DMA prefetch margin
K_PF = int(os.environ.get("K_PF", "14"))  # max blocks of early chunk DMA
K_BT = int(os.environ.get("K_BT", "4"))   # bt-bank buffers
K_WK = int(os.environ.get("K_WK", "4"))   # work-bank buffers
K_FLUSH_EARLY = int(os.environ.get("K_FLUSH_EARLY", "0"))  # flush next copy's
# sources before this block's L@y1 (overlap ACT latency) instead of after
K_PACK = int(os.environ.get("K_PACK", "0"))  # wa+wb share one PSUM bank
K_DEEP = int(os.environ.get("K_DEEP", "0"))  # prework 2 blocks ahead
K_PRE_EARLY = int(os.environ.get("K_PRE_EARLY", "0"))  # prework before L@y1
K_2ACT = int(os.environ.get("K_2ACT", "1"))  # drop the y0 seed ACT (2 ACTs/blk)
CHUNK = int(os.environ.get("K_CHUNK", str(CHUNK)))

_CACHE = {}


def _grp_cw(g):
    return 128 * min(GROUP, NBLK - GROUP * g)


def _grp_dmax(g):
    return min(GROUP * g + GROUP - 1, NBLK - 1)


def _grp_kt(g):
    return KX + _grp_dmax(g) + 1


def _grp_full(g):
    return _grp_cw(g) == 512


def _grp_ktm(g):
    """Main-panel rows: full groups push their last 3 (mostly unused) rows
    into a compact 'wd' strip; the last narrow group keeps everything."""
    return KX + GROUP * g + 1 if _grp_full(g) else _grp_kt(g)


def _grp_chunks(g):
    kt_n = _grp_ktm(g)
    return [(c0, min(c0 + CHUNK, kt_n)) for c0 in range(0, kt_n, CHUNK)]


def _grp_units(g):
    """Stream-emission units: ('p', kt0) = DoubleRow pair (kt0, kt0+1),
    ('s', kt) = single, ('t', kt) = trimmed single (wd strip).
    Pairs are even-aligned so they never straddle an 8-row chunk."""
    units = [("p", 2 * i) for i in range(KX // 2)]
    if _grp_full(g):
        units += [("p", KX + 2 * i) for i in range(2 * g)]   # Y s=0..4g-1
        units += [("s", KX + 4 * g), ("t", KX + 4 * g + 1)]
    else:
        smax = _grp_dmax(g) - 2
        units += [("p", KX + 2 * i) for i in range(smax // 2)]
        units += [("s", KX + smax)]
    return units


# wd strip layout (full groups): [row KX+4g+1 cols 128:512 | row KX+4g+2
# cols 256:512 | row KX+4g+3 cols 384:512] -> local offsets 0/384/640, 768 wide
WD_W = 768


def _build_module():
    import concourse.mybir as mybir
    import concourse.tile as tile
    from concourse import bacc
    from concourse.bass import ds, ts
    from concourse.masks import make_identity
    from contextlib import ExitStack

    bf = mybir.dt.bfloat16
    f8 = mybir.dt.float8e4
    f32 = mybir.dt.float32
    Tanh = mybir.ActivationFunctionType.Tanh
    Sigmoid = mybir.ActivationFunctionType.Sigmoid
    DR = mybir.MatmulPerfMode.DoubleRow
    INV = 1.0 / WSCALE

    nc = bacc.Bacc()
    x_in = nc.dram_tensor("xt", [128, KX, BL], f8, kind="ExternalInput")
    w_in = {}
    wd_in = {}
    for g in range(NGRP):
        cw = _grp_cw(g)
        for ci, (k0, k1) in enumerate(_grp_chunks(g)):
            w_in[(g, ci)] = nc.dram_tensor(
                f"w{g}_{ci}", [128, k1 - k0, cw], f8, kind="ExternalInput"
            )
        if _grp_full(g):
            wd_in[g] = nc.dram_tensor(f"wd{g}", [128, WD_W], f8,
                                      kind="ExternalInput")
    out_t = nc.dram_tensor("out", [128, BL], f32, kind="ExternalOutput")

    with ExitStack() as ctx:
        tc = ctx.enter_context(tile.TileContext(nc))
        singles = ctx.enter_context(tc.tile_pool(name="singles", bufs=1))
        panels = ctx.enter_context(tc.tile_pool(name="panels", bufs=20))
        psum = ctx.enter_context(tc.tile_pool(name="psum", bufs=3, space="PSUM"))
        chain = ctx.enter_context(tc.tile_pool(name="chain", bufs=4))

        xt = singles.tile([128, KX, BL], f8)
        nc.sync.dma_start(out=xt, in_=x_in[:])
        ident = singles.tile([BL, BL], f32)
        make_identity(nc, ident)
        yall = singles.tile([128, NBLK, BL], f8)

        banks = {}     # g -> psum tile [64, cw]
        ptiles = {}    # (g, kt) -> (tile, local_kt)
        started = set()  # banks whose start=True matmul was emitted
        pending = {}   # g -> list of emission units not yet emitted
        alloc_hi = -1  # highest allocated group

        def pt(g, kt):
            t, lk = ptiles[(g, kt)]
            return t[:, lk, :]

        chunk_meta = {}  # g -> [(ci, k0, k1), ...] not yet DMA'd

        def alloc_group(g):
            banks[g] = psum.tile([64, _grp_cw(g)], f32, tag="bt", bufs=K_BT,
                                 name=f"bank{g}")
            pending[g] = _grp_units(g)

        wdt = {}  # g -> wd strip tile [128, 768]
        for g in range(NGRP):
            chunk_meta[g] = list(enumerate(_grp_chunks(g)))
            if _grp_full(g):
                chunk_meta[g].append(("wd", (KX + GROUP * g + 1, 0)))

        def pump_dma(d):
            """JIT panel loads, decoupled from bank allocation: a chunk's DMA
            is emitted ~K_MARGIN blocks before its sources become available
            (but no earlier than K_PF blocks before its group starts), so
            late groups' bulk streams early and the post-DMA tail is short."""
            for g in sorted(chunk_meta):
                rest = []
                for ci, (k0, k1) in chunk_meta[g]:
                    if d < max(k0 - KX - K_MARGIN, GROUP * g - K_PF):
                        rest.append((ci, (k0, k1)))
                    elif ci == "wd":
                        wtile = panels.tile([128, WD_W], f8, tag="wd", bufs=4,
                                            name=f"wd{g}")
                        nc.sync.dma_start(out=wtile, in_=wd_in[g][:])
                        wdt[g] = wtile
                    else:
                        cw = _grp_cw(g)
                        ptile = panels.tile(
                            [128, k1 - k0, cw], f8, tag=f"pan{cw}",
                            bufs=(20 if cw == 512 else 6),
                            name=f"p{g}_{ci}",
                        )
                        nc.sync.dma_start(out=ptile, in_=w_in[(g, ci)][:])
                        for kk in range(k0, k1):
                            ptiles[(g, kk)] = (ptile, kk - k0)
                if rest:
                    chunk_meta[g] = rest
                else:
                    del chunk_meta[g]

        def ldiag_ap(d):
            g, dc = d // GROUP, d % GROUP
            if not _grp_full(g) or dc == 0:
                return pt(g, KX + d)[:, ts(dc, 128)]
            return wdt[g][:, ds((0, 384, 640)[dc - 1], 128)]

        def wprev_ap(d):
            g, dc = d // GROUP, d % GROUP  # row KX+d-1, cols dc*128:+128
            if not _grp_full(g) or dc <= 1:
                return pt(g, KX + d - 1)[:, ts(dc, 128)]
            return wdt[g][:, ds((128, 512)[dc - 2], 128)]

        def emit_stream(g, u):
            kind, kt = u
            first = g not in started
            if first:
                started.add(g)
            last_kt = KX + _grp_dmax(g) - 2
            if kind == "t":
                # trimmed last source: only its dest-block-3 columns exist
                nc.tensor.matmul(
                    banks[g][:, ds(384, 128)], lhsT=yall[:, kt - KX, :],
                    rhs=wdt[g][:, ds(256, 128)], start=first,
                    stop=kt == last_kt,
                )
            elif kind == "s":
                lhsT = xt[:, kt, :] if kt < KX else yall[:, kt - KX, :]
                nc.tensor.matmul(
                    banks[g], lhsT=lhsT, rhs=pt(g, kt), start=first,
                    stop=kt == last_kt,
                )
            else:  # DoubleRow pair (kt, kt+1); never the stop unit
                lhsT = (xt[:, ds(kt, 2), :] if kt < KX
                        else yall[:, ds(kt - KX, 2), :])
                t, lk = ptiles[(g, kt)]
                t2, lk2 = ptiles[(g, kt + 1)]
                assert t is t2 and lk2 == lk + 1
                nc.tensor.matmul(
                    banks[g], lhsT=lhsT, rhs=t[:, ds(lk, 2), :],
                    start=first, stop=False, perf_mode=DR,
                )

        def can_emit(u, d):
            kind, kt0 = u
            kt_last = kt0 + 1 if kind == "p" else kt0
            return kt_last < KX or kt_last - KX <= d - 1

        def tiles_ready(g, u):
            kind, kt0 = u
            if kind == "t":
                return g in wdt
            if kind == "p":
                return (g, kt0) in ptiles and (g, kt0 + 1) in ptiles
            return (g, kt0) in ptiles

        def flush(g, d):
            """Emit all pending source MMs for bank g allowed at iter d,
            splitting a pair whose second half is past the frontier."""
            keep = []
            for u in pending[g]:
                if can_emit(u, d):
                    emit_stream(g, u)
                elif u[0] == "p" and u[1] - KX <= d - 1:
                    emit_stream(g, ("s", u[1]))
                    keep.append(("s", u[1] + 1))
                else:
                    keep.append(u)
            pending[g] = keep

        def drip(d, k):
            for g in sorted(pending):
                while pending[g] and k > 0:
                    u = pending[g][0]
                    if not can_emit(u, d) or not tiles_ready(g, u):
                        break
                    pending[g].pop(0)
                    emit_stream(g, u)
                    k -= 1

        def prework(d):
            """Copy+transpose block d's base slice (and in 3-ACT mode seed
            y0 = tanh(partial base) a block early)."""
            g, dc = d // GROUP, d % GROUP
            flush(g, d - 1)  # slice d needs sources <= d-2 (emitted <= iter d-1)
            sb_bt = chain.tile([64, 128], f32, tag="sbt")
            nc.vector.tensor_copy(sb_bt, banks[g][:, ts(dc, 128)])
            if K_PACK:
                wt = psum.tile([128, 2, BL], f32, tag="wk", bufs=K_WK,
                               name=f"w{d}")
                wa, wb = wt[:, 0, :], wt[:, 1, :]
            else:
                wa = psum.tile([128, BL], f32, tag="wk", bufs=K_WK, name=f"wa{d}")
                wb = psum.tile([128, BL], f32, tag="wk", bufs=K_WK, name=f"wb{d}")
            # in 2-ACT mode block 0's wa has no wprev MM: close it here
            nc.tensor.matmul(wa, lhsT=sb_bt, rhs=ident, is_transpose=True,
                             start=True, stop=bool(K_2ACT and d == 0))
            nc.tensor.matmul(wb, lhsT=sb_bt, rhs=ident, is_transpose=True,
                             start=True, stop=False)
            if K_2ACT:
                return wa, wb, None
            y0 = chain.tile([128, BL], bf, tag="yc", bufs=8, name=f"y0_{d}")
            nc.scalar.activation(out=y0, in_=wa, func=Tanh, scale=INV)
            return wa, wb, y0

        state = {}  # d -> (wa, wb, y0)
        for g in range(min((LOOKAHEAD + GROUP - 1) // GROUP + 1, NGRP)):
            alloc_group(g)
            alloc_hi = g
        pump_dma(0)
        state[0] = prework(0)
        if K_DEEP and NBLK > 1:
            state[1] = prework(1)  # block 1's base needs only x streams

        for d in range(NBLK):
            g, dc = d // GROUP, d % GROUP
            want = min((d + LOOKAHEAD) // GROUP, NGRP - 1)
            while alloc_hi < want:
                alloc_hi += 1
                alloc_group(alloc_hi)
            pump_dma(d)
            wa, wb, y0 = state.pop(d)
            ldiag = ldiag_ap(d)  # strictly-lower masked on host
            # ---- critical path: two decoupled one-ACT chains ----
            # 3-ACT: A_d += W_prev @ y1_{d-1} (stale) + L @ y0 -> y1 = tanh(A)
            # 2-ACT: A_d += W_prev @ y1_{d-1} (no L seed)  -> y1 = tanh(A)
            # y2-chain: B_d += W_prev @ y2_{d-1} (exact) + L @ y1 -> y2 = tanh(B)
            if d > 0:
                wprev = wprev_ap(d)
                nc.tensor.matmul(wa, lhsT=wprev, rhs=y1_prev, start=False,
                                 stop=bool(K_2ACT))
            if not K_2ACT:
                nc.tensor.matmul(wa, lhsT=ldiag, rhs=y0, start=False, stop=True)
            y1 = chain.tile([128, BL], bf, tag="yc", bufs=8, name=f"y1_{d}")
            nc.scalar.activation(out=y1, in_=wa, func=Tanh, scale=INV)
            if d > 0:
                nc.tensor.matmul(wb, lhsT=wprev, rhs=yall[:, d - 1, :],
                                 start=False, stop=False)
            if K_FLUSH_EARLY and d + 1 < NBLK:
                flush((d + 1) // GROUP, d)  # next copy's sources, under ACT(y1)
            if K_PRE_EARLY and d + 1 < NBLK and d + 1 not in state:
                # prework before L@y1: ACT order becomes [y1_d, y0_{d+1},
                # y2_d] so y0 no longer sits between y2_d and y1_{d+1}
                state[d + 1] = prework(d + 1)
            drip(d, K_DRIP1)
            nc.tensor.matmul(wb, lhsT=ldiag, rhs=y1, start=False, stop=True)
            y1_prev = y1
            if d < NBLK - 1:
                nc.scalar.activation(out=yall[:, d, :], in_=wb, func=Tanh,
                                     scale=INV)
            else:
                yfin = chain.tile([128, BL], f32, tag="yf")
                nc.scalar.activation(out=yfin, in_=wb, func=Tanh, scale=INV)
                # sigmoid(y) = 0.5*tanh(y/2) + 0.5 — same Tanh table, no
                # 1.3us Sigmoid LoadActFuncSet on the critical tail.
                th = chain.tile([128, BL], f32, tag="of")
                nc.scalar.activation(out=th, in_=yfin, func=Tanh, scale=0.5)
                ofin = chain.tile([128, BL], f32, tag="og")
                nc.vector.tensor_scalar(
                    out=ofin, in0=th, scalar1=0.5, scalar2=0.5,
                    op0=mybir.AluOpType.mult, op1=mybir.AluOpType.add,
                )
                nc.sync.dma_start(out=out_t[:], in_=ofin)
            # ---- pre-work for a later block (overlaps this block's tail) ----
            nxt = d + 2 if K_DEEP else d + 1
            if nxt < NBLK and nxt not in state:
                state[nxt] = prework(nxt)
                drip(d, K_DRIP2)
            elif d + 1 < NBLK:
                drip(d, K_DRIP2)
    nc.compile()
    return nc


def _get_module():
    if "nc" not in _CACHE:
        _CACHE["nc"] = _build_module()
    return _CACHE["nc"]


_STRICT_LOWER = (np.arange(NB)[:, None] < np.arange(NB)[None, :]).astype(np.float32)


def _pack_w(W):
    """Group panels: pan[p, kt, c] = W[512*g + c, kt*128 + p], e4m3*WSCALE,
    chunked. Each group's diagonal 128x128 sub-tiles are masked
    strictly-lower before quantization."""
    maps = {}
    W = np.asarray(W, np.float32)
    for g in range(NGRP):
        cw = _grp_cw(g)
        kt_n = _grp_kt(g)
        c0 = 512 * g
        blk = W[c0 : c0 + cw, : kt_n * 128]          # [c, kt*128]
        pan = np.ascontiguousarray(
            blk.reshape(cw, kt_n, 128).transpose(2, 1, 0)
        )                                             # [p, kt, c]
        for dc in range(cw // 128):
            d = GROUP * g + dc
            pan[:, KX + d, dc * 128 : (dc + 1) * 128] *= _STRICT_LOWER
        pan = (pan * WSCALE).astype(F8E4)
        for ci, (k0, k1) in enumerate(_grp_chunks(g)):
            maps[f"w{g}_{ci}"] = np.ascontiguousarray(pan[:, k0:k1, :])
        if _grp_full(g):
            r = KX + GROUP * g + 1
            maps[f"wd{g}"] = np.ascontiguousarray(
                np.concatenate(
                    [pan[:, r, 128:], pan[:, r + 1, 256:], pan[:, r + 2, 384:]],
                    axis=1,
                )
            )
    return maps


def _pack_x(xs):
    """xt[p, kt, c] = xs[c, kt*128 + p], e4m3 (unit scale). xs: [BL, IN]."""
    return np.ascontiguousarray(
        np.asarray(xs, np.float32).reshape(BL, KX, 128).transpose(2, 1, 0)
    ).astype(F8E4)


def kernel(x, W, output_size=OUT):
    from concourse.bass_utils import run_bass_kernel_spmd

    assert int(output_size) == OUT
    x = np.asarray(x, np.float32)
    assert x.shape == (B, IN) and np.asarray(W).shape == (NN, IN + NN)

    nc = _get_module()
    wmaps = _pack_w(W)
    in_maps = [
        {"xt": _pack_x(x[ci * BL : (ci + 1) * BL]), **wmaps} for ci in range(NCORES)
    ]
    res = run_bass_kernel_spmd(nc, in_maps, core_ids=list(range(NCORES)))
    out = np.empty((B, OUT), np.float32)
    for ci in range(NCORES):
        out[ci * BL : (ci + 1) * BL] = res.results[ci]["out"].T
    return out
